# revision 1
# baseline (speedup 1.0000x reference)
"""LinearKAN (Gaussian-RBF KAN layer) Trainium2 kernel.

Math (per reference):
    phi[b,a,i] = exp(-((x[b,i] - g_a)/h)^2)         g = linspace(-2, 2, 8), h = 4/7
    out[b,o]   = sum_{a,i} phi[b,a,i] * (c[a,o,i]*w_s[o,i])  +  sum_i silu(x[b,i]) * w_b[o,i]

Strategy: data-parallel over the batch across 8 NeuronCores. Each core:
  - folds W[k=(a,i), o] = c^T * w_s^T on DVE (bf16), appends w_b^T as 6 extra
    k-tiles so the residual rides the same PSUM accumulation,
  - computes phi^T[(a,i), b] tiles on ACT (Square with fused affine, then Exp),
    and silu(x)^T tiles (ACT Silu), cast to bf16,
  - accumulates out^T[o, b] = W^T @ phi in PSUM over 54 k-tiles of 128,
  - copies PSUM -> SBUF -> HBM (fp32).
Host transposes x per shard / c / w_s / w_b on the way in and out^T on the way
out (layout only; all arithmetic is on-device).
"""

import ml_dtypes
import numpy as np

import concourse.bacc as bacc
import concourse.tile as tile
from concourse import mybir
from concourse.bass_utils import run_bass_kernel_spmd

N_CORES = 8
BATCH, IN_F, OUT_F = 16384, 768, 768
B_SHARD = BATCH // N_CORES          # 2048
GRID_SIZE, GRID_LO, GRID_HI = 8, -2.0, 2.0
H = (GRID_HI - GRID_LO) / (GRID_SIZE - 1)
P = 128
I_TILES = IN_F // P                 # 6
O_TILES = OUT_F // P                # 6
K_SPLINE = GRID_SIZE * I_TILES      # 48 k-tiles for the spline contraction
K_TOTAL = K_SPLINE + I_TILES        # +6 k-tiles for the silu residual
B_TILE = 512
N_BTILES = B_SHARD // B_TILE        # 4

F32 = mybir.dt.float32
BF16 = mybir.dt.bfloat16
AF = mybir.ActivationFunctionType

# Fraction of the Square (z^2) passes routed to DVE instead of ACT, to balance
# the two engines. a-index < SQUARE_ON_DVE_A[bt] go to DVE. b_tile 0 leans on
# ACT because DVE is busy with the one-time W-fold during it.
SQUARE_ON_DVE_A = (2, 4, 4, 4)


def _build_nc():
    nc = bacc.Bacc(None, target_bir_lowering=False, debug=False)

    xT = nc.dram_tensor("xT", [IN_F, B_SHARD], F32, kind="ExternalInput")
    c_t = nc.dram_tensor("c_t", [GRID_SIZE, IN_F, OUT_F], BF16, kind="ExternalInput")
    wsT = nc.dram_tensor("wsT", [IN_F, OUT_F], F32, kind="ExternalInput")
    wbT = nc.dram_tensor("wbT", [IN_F, OUT_F], BF16, kind="ExternalInput")
    outT = nc.dram_tensor("outT", [OUT_F, B_SHARD], F32, kind="ExternalOutput")

    xT_ap = xT.ap()
    c_ap = c_t.ap()
    wsT_ap = wsT.ap()
    wbT_ap = wbT.ap()
    outT_ap = outT.ap()

    grid = np.linspace(GRID_LO, GRID_HI, GRID_SIZE, dtype=np.float64)

    with tile.TileContext(nc) as tc:
        with (
            tc.tile_pool(name="wpool", bufs=1) as wpool,
            tc.tile_pool(name="wspool", bufs=1) as wspool,
            tc.tile_pool(name="cstage", bufs=4) as cstage,
            tc.tile_pool(name="xpool", bufs=12) as xpool,
            tc.tile_pool(name="phipool", bufs=14) as phipool,
            tc.tile_pool(name="sqpool", bufs=4) as sqpool,
            tc.tile_pool(name="opool", bufs=8) as opool,
            tc.tile_pool(name="psum", bufs=8, space="PSUM") as psum_pool,
        ):
            # ---- PE warmup: dummy matmuls during the initial DMA window so
            # the HAM clock gate reaches 8/8 (2.4 GHz) before the real MM
            # stream starts (saves the ~3.4us cold ramp on real work) ----
            wa = wspool.tile([P, P], BF16, tag="warm_a", name="warm_a")
            nc.vector.memset(wa, 0.0)
            wb_ = wspool.tile([P, B_TILE], BF16, tag="warm_b", name="warm_b")
            nc.vector.memset(wb_, 0.0)
            wp = psum_pool.tile([P, B_TILE], F32, tag="ps", name="warm_ps")
            for i in range(12):
                nc.tensor.matmul(wp, wa, wb_, start=(i == 0), stop=(i == 11))

            # ---- per-a bias tiles for the ACT Square affine: -g_a / h ----
            bias_tiles = []
            for a in range(GRID_SIZE):
                bt_ = wspool.tile([P, 1], F32, tag=f"bias{a}", name=f"bias{a}")
                nc.vector.memset(bt_, float(-grid[a] / H))
                bias_tiles.append(bt_)

            # ---- prefetch b_tile 0's first x tile ahead of everything so the
            # first phi tile (and matmul) has the shortest dependency chain;
            # the remaining x tiles follow the first ws/c pair ----
            x_tiles_bt0 = []
            for it in range(I_TILES):
                xt = xpool.tile([P, B_TILE], F32, tag="x", name=f"x0_{it}")
                x_tiles_bt0.append(xt)
            nc.sync.dma_start(out=x_tiles_bt0[0], in_=xT_ap[0:P, 0:B_TILE])

            # ---- fold W[k] = bf16(c^T * w_s^T); k = it*GRID_SIZE + a (i-major
            # so each ws tile is consumed right after its DMA, keeping the
            # first matmul's dependency chain short) ----
            w_tiles = [None] * K_TOTAL
            for it in range(I_TILES):
                wsf = wspool.tile([P, OUT_F], F32, tag="wsf", bufs=2, name=f"wsf{it}")
                nc.sync.dma_start(out=wsf, in_=wsT_ap[it * P:(it + 1) * P, :])
                wst = wspool.tile([P, OUT_F], BF16, tag="ws", bufs=2, name=f"ws{it}")
                nc.vector.tensor_copy(wst, wsf)
                for a in range(GRID_SIZE):
                    k = it * GRID_SIZE + a
                    ct = cstage.tile([P, OUT_F], BF16, tag="cstage", bufs=8,
                                     name=f"c{k}")
                    nc.sync.dma_start(out=ct, in_=c_ap[a, it * P:(it + 1) * P, :])
                    wt = wpool.tile([P, OUT_F], BF16, tag=f"w{k}", name=f"w{k}")
                    nc.vector.tensor_mul(wt, ct, wst)
                    w_tiles[k] = wt
                if it == 0:
                    # rest of b_tile 0's x right after the first ws/c pair
                    for it2 in range(1, I_TILES):
                        nc.sync.dma_start(
                            out=x_tiles_bt0[it2],
                            in_=xT_ap[it2 * P:(it2 + 1) * P, 0:B_TILE])
            # residual weights: W[K_SPLINE + j] = bf16(0.5 * w_b^T tile j).
            # The 0.5 compensates silu(x) = 0.5*x*(1 + tanh(x/2)) being fed to
            # the PE as s = x + x*tanh(x/2) (tanh shares the exp ACT table set,
            # avoiding per-btile table switches that Silu would cause).
            for j in range(I_TILES):
                k = K_SPLINE + j
                ct = cstage.tile([P, OUT_F], BF16, tag="cstage", bufs=8,
                                 name=f"wb{j}")
                nc.sync.dma_start(out=ct, in_=wbT_ap[j * P:(j + 1) * P, :])
                wt = wpool.tile([P, OUT_F], BF16, tag=f"w{k}", name=f"w{k}")
                nc.vector.tensor_scalar_mul(wt, ct, 0.5)
                w_tiles[k] = wt

            # ---- main loop over batch tiles ----
            for bt in range(N_BTILES):
                bsl = slice(bt * B_TILE, (bt + 1) * B_TILE)
                if bt == 0:
                    x_tiles = x_tiles_bt0
                else:
                    x_tiles = []
                    for it in range(I_TILES):
                        xt = xpool.tile([P, B_TILE], F32, tag="x", name=f"x{bt}_{it}")
                        nc.sync.dma_start(out=xt, in_=xT_ap[it * P:(it + 1) * P, bsl])
                        x_tiles.append(xt)

                psums = []
                for o in range(O_TILES):
                    ps = psum_pool.tile([P, B_TILE], F32, tag="ps", name=f"ps{bt}_{o}")
                    psums.append(ps)

                for k in range(K_TOTAL):
                    ph = phipool.tile([P, B_TILE], BF16, tag="phi", name=f"phi{bt}_{k}")
                    if k < K_SPLINE:
                        it, a = divmod(k, GRID_SIZE)
                        g = float(grid[a])
                        sq = sqpool.tile([P, B_TILE], F32, tag="sq", name=f"sq{bt}_{k}")
                        if a < SQUARE_ON_DVE_A[bt]:
                            # z = (x - g)/h on DVE (2x fp32), z*z on DVE (1x)
                            z = sqpool.tile([P, B_TILE], F32, tag="z", name=f"z{bt}_{k}")
                            nc.vector.tensor_scalar(
                                out=z, in0=x_tiles[it],
                                scalar1=g, scalar2=1.0 / H,
                                op0=mybir.AluOpType.subtract,
                                op1=mybir.AluOpType.mult,
                            )
                            nc.vector.tensor_mul(sq, z, z)
                        else:
                            # z^2 = Square(x/h - g/h) on ACT
                            nc.scalar.activation(
                                out=sq, in_=x_tiles[it], func=AF.Square,
                                bias=bias_tiles[a], scale=1.0 / H,
                            )
                        # phi = exp(-z^2), cast to bf16
                        nc.scalar.activation(out=ph, in_=sq, func=AF.Exp, scale=-1.0)
                    else:
                        # s = x*(1 + tanh(x/2))  (= 2*silu(x); W carries the 0.5)
                        it = k - K_SPLINE
                        th = sqpool.tile([P, B_TILE], F32, tag="sq", name=f"th{bt}_{k}")
                        nc.scalar.activation(out=th, in_=x_tiles[it], func=AF.Tanh,
                                             scale=0.5)
                        nc.vector.scalar_tensor_tensor(
                            out=ph, in0=th, scalar=1.0, in1=x_tiles[it],
                            op0=mybir.AluOpType.add, op1=mybir.AluOpType.mult,
                        )

                    for o in range(O_TILES):
                        nc.tensor.matmul(
                            psums[o],
                            w_tiles[k][:, o * P:(o + 1) * P],
                            ph,
                            start=(k == 0),
                            stop=(k == K_TOTAL - 1),
                        )

                for o in range(O_TILES):
                    ot = opool.tile([P, B_TILE], F32, tag="out", name=f"out{bt}_{o}")
                    # alternate PSUM-drain engines so the end-of-btile copies
                    # pipeline two at a time
                    if o % 2 == 0:
                        nc.vector.tensor_copy(ot, psums[o])
                    else:
                        nc.scalar.copy(ot, psums[o])
                    nc.sync.dma_start(out=outT_ap[o * P:(o + 1) * P, bsl], in_=ot)

    nc.compile()
    return nc


_NC_CACHE = {}


def _get_nc():
    if "nc" not in _NC_CACHE:
        _NC_CACHE["nc"] = _build_nc()
    return _NC_CACHE["nc"]


def kernel(x, w_b, w_s, c):
    x = np.ascontiguousarray(np.asarray(x, dtype=np.float32))
    w_b = np.ascontiguousarray(np.asarray(w_b, dtype=np.float32))
    w_s = np.ascontiguousarray(np.asarray(w_s, dtype=np.float32))
    c = np.ascontiguousarray(np.asarray(c, dtype=np.float32))

    xT = np.ascontiguousarray(x.T)                      # [IN_F, BATCH]
    c_t = np.ascontiguousarray(
        c.transpose(0, 2, 1)).astype(ml_dtypes.bfloat16)    # [a, i, o]
    wsT = np.ascontiguousarray(w_s.T)                   # [i, o]
    wbT = np.ascontiguousarray(w_b.T).astype(ml_dtypes.bfloat16)  # [i, o]

    in_maps = []
    for ci in range(N_CORES):
        in_maps.append({
            "xT": np.ascontiguousarray(xT[:, ci * B_SHARD:(ci + 1) * B_SHARD]),
            "c_t": c_t,
            "wsT": wsT,
            "wbT": wbT,
        })

    res = run_bass_kernel_spmd(_get_nc(), in_maps, core_ids=list(range(N_CORES)))
    outT = np.concatenate([r["outT"] for r in res.results], axis=1)  # [OUT_F, BATCH]
    return np.ascontiguousarray(outT.T).astype(np.float32, copy=False)


if __name__ == "__main__":
    rng = np.random.default_rng(0)
    x = rng.standard_normal((BATCH, IN_F), dtype=np.float32)
    w_b = rng.standard_normal((OUT_F, IN_F), dtype=np.float32) * 1e-3
    w_s = np.ones((OUT_F, IN_F), dtype=np.float32)
    c = (rng.standard_normal((GRID_SIZE, OUT_F, IN_F)) * 1e-3).astype(np.float32)
    out = kernel(x, w_b, w_s, c)
    print(out.shape, out.dtype)

